# revision 28
# baseline (speedup 1.0000x reference)
"""Self-contained Trainium2 (Bass/Tile) multi-head attention kernel.

Problem: nn_MultiHeadAttention (B=4, T=2048, C=1024, H=16 heads, D=64),
fp32, causal, torch-Linear-style projections (y = x @ W.T + b).

Sharding (8 NeuronCores): data-parallel over B (4) x tensor-parallel over
head-groups (2 groups of 8 heads). Core c handles batch c//2, head group
c%2. Each core computes Q/K/V projections for its 512 features, causal
attention for its 8 heads, and a partial output projection
(O_group @ Wo[:, group].T). The host sums the two partials per batch and
adds bo.

v4 structure (measured rationale in comments):
  - Projections stay bf16: a DoubleRow fp8 matmul streams output columns
    at the same 1 col/cycle as bf16 (it doubles contraction per *pass*,
    not column rate), and the dual-fp8 residual scheme needs 12 passes
    vs bf16's 8 -- a net loss. bf16 LDWEIGHTS hide behind the previous
    matmul via the PE's background weight buffer.
  - PV runs fp8 DoubleRow over 256-token k-pairs for q-tiles >= 1: one
    240ns DR matmul replaces two 216ns bf16 ones (contraction 256 in one
    pass), with the V+ones stationary keeping softmax denominators in
    PSUM row 64. Q-tile 0 stays bf16: early queries average few keys, so
    fp8 P/V noise doesn't average out there (fp8 anywhere else that
    feeds a long random-walk sum would land ~3.6% error on the output).
  - The attention loop is q-tile-major (qt, then head-chunk): projection
    filler deadlines then spread across the whole span instead of
    piling into chunk 0, which keeps TensorE dense enough that the HAM
    clock gate stays at full rate; and each q-tile's output projection
    overlaps the next q-tile's attention.
  - Softmax normalization: r = exp(-ln(sums)) on ScalarE (one table set
    has both; the DVE reciprocal ops fail codegen on this walrus),
    partition-broadcast via a DRAM bounce, applied on DVE. The
    multiplies are deferred one attention cell so the strict-FIFO DVE
    never parks on the bounce DMA semaphore and starves the PE.
  - Causal masks are added in place on the S PSUM over exactly the
    128/256 columns that need them (maskA = tri, maskB = full|tri for
    the odd half of a diagonal k-pair).

On-device layout (per core), nothing ever needs an on-chip transpose:
  - Q^T, K^T: [feat 512, tok T] bf16 (head pair per 128-chunk); the two
    heads' S matmuls run concurrently via PE row tiling (K=64).
  - P^T = exp(0.125 * S^T + mask) (ScalarE) -> fp8 pair tile / bf16
  - PV: [65, q] += matmul(lhsT=V_aug, rhs=P^T): rows 0..63 =
    unnormalized O^T, row 64 = softmax denominators
  - out partial [tok, C] = matmul(lhsT=O^T chunks, rhs=Wo_g^T chunks)
"""

import numpy as np
import ml_dtypes

import bass_rust
import concourse.bass as bass
import concourse.mybir as mybir
import concourse.tile as tile
from concourse.bass_utils import run_bass_kernel_spmd
from concourse.vector_clock import ScopedClock

BF16 = ml_dtypes.bfloat16

B, T, C, H, D = 4, 2048, 1024, 16, 64
G = C // 2          # features per head group (8 heads x 64)
N_CORES = 8
MASK_NEG = -800.0   # pre-scale; exp(0.125 * (s - 800)) == 0 for |s| < 30

# ---------------------------------------------------------------------------
# The walrus build in this container rejects instructions carrying more than
# a couple of sync waits ("Too many sync wait commands"). Tile's kernel-tail
# drain aggregates one wait per live semaphore. Split them into individual
# SP wait instructions (program order on SP preserves the semantics).
# ---------------------------------------------------------------------------


def _patched_drain_and_barrier(self, tick_clock, wait_clock):
    nc = self.nc
    drain_inst = nc.sync.drain()
    wait_clock.add_sem_waits(
        drain_inst.ins, ScopedClock({None: tick_clock.global_clock})
    )
    si = drain_inst.ins.sync_info
    waits = list(si.on_wait) if si is not None else []
    if waits:
        drain_inst.ins.sync_info = bass_rust.SyncInfo(
            on_wait=[], on_update=list(si.on_update)
        )
        assert self.sems is not None
        by_name = {h.name: h for h in self.sems.allocated().values()}
        for w in waits:
            assert w.wait_mode == "sem-ge-imm", w
            nc.sync.wait_ge(by_name[w.ant_name], w.wait_value)

    nc.all_engine_barrier()
    assert self.sems is not None
    popped = nc._tile_sem_poison_stack.pop()
    assert popped is self._sem_poison
    nc.clear_and_free_semaphores(list(self.sems.allocated().values()))
    nc.all_engine_barrier()


tile.TileContext._drain_and_barrier = _patched_drain_and_barrier


def _split_excess_waits(nc, max_waits=1):
    """Hoist surplus sync waits into standalone same-engine EventSemaphore
    instructions placed right before the owner (this walrus encodes at most
    one wait per instruction)."""
    n = 0
    for fn in nc.m.functions:
        for blk in fn.blocks:
            new_insts = []
            for inst in blk.instructions:
                si = inst.sync_info
                waits = list(si.on_wait) if si is not None else []
                if len(waits) > max_waits:
                    for w in waits[:-max_waits]:
                        ev = mybir.InstEventSemaphore(
                            name=f"I-wsplit-{n}", ins=[], outs=[]
                        )
                        n += 1
                        ev.engine = inst.engine
                        ev.sync_info = bass_rust.SyncInfo(
                            on_wait=[w], on_update=[]
                        )
                        new_insts.append(ev)
                    inst.sync_info = bass_rust.SyncInfo(
                        on_wait=waits[-max_waits:], on_update=list(si.on_update)
                    )
                new_insts.append(inst)
            blk.instructions = new_insts


# ---------------------------------------------------------------------------
# Kernel builder (per-core program; same program on all 8 cores)
# ---------------------------------------------------------------------------

def build_nc(t=T, split_waits=True):
    f32 = mybir.dt.float32
    bf16 = mybir.dt.bfloat16
    f8 = mybir.dt.float8e4
    Exp = mybir.ActivationFunctionType.Exp
    DR = mybir.MatmulPerfMode.DoubleRow

    assert t % 512 == 0
    TS = t // 512            # 512-token slices (q-tiles)
    TK = t // 128            # 128-token k-tiles
    KP = TK // 2             # 256-token k-pairs

    nc = bass.Bass()
    xt_d = nc.dram_tensor("xt", [C, t], bf16, kind="ExternalInput")
    wqt_d = nc.dram_tensor("wqt", [C, G], bf16, kind="ExternalInput")
    wkt_d = nc.dram_tensor("wkt", [C, G], bf16, kind="ExternalInput")
    wvt_d = nc.dram_tensor("wvt", [C, G], bf16, kind="ExternalInput")
    wot_d = nc.dram_tensor("wot", [G, C], bf16, kind="ExternalInput")
    bqk_d = nc.dram_tensor("bqk", [128, 8], f32, kind="ExternalInput")
    bv_d = nc.dram_tensor("bv", [G], f32, kind="ExternalInput")
    # maskA = tri(128); maskB = [full -800 (128) | tri (128)]
    maskA_d = nc.dram_tensor("maskA", [128, 128], f32, kind="ExternalInput")
    maskB_d = nc.dram_tensor("maskB", [128, 256], f32, kind="ExternalInput")
    out_d = nc.dram_tensor("out", [t, C], f32, kind="ExternalOutput")
    rsc_d = nc.dram_tensor("rscratch", [16, 512], bf16, kind="ExternalOutput")

    with tile.TileContext(nc) as tc:
        with (
            tc.tile_pool(name="big", bufs=1) as big,
            tc.tile_pool(name="weights", bufs=1) as wpool,
            tc.tile_pool(name="xsl", bufs=3) as xpool,
            tc.tile_pool(name="pt", bufs=4) as ptpool,
            tc.tile_pool(name="ptb", bufs=3) as ptbpool,
            tc.tile_pool(name="small", bufs=4) as small,
            tc.tile_pool(name="psA", bufs=2, space="PSUM") as psA,
            tc.tile_pool(name="psS", bufs=2, space="PSUM") as psS,
            tc.tile_pool(name="psPV", bufs=2, space="PSUM") as psPV,
        ):
            # ---- persistent SBUF tensors ----
            qt_sb = big.tile([128, 4, t], bf16, tag="qt")      # Q^T
            kt_sb = big.tile([128, 4, t], bf16, tag="kt")      # K^T
            # fp8 V + ones, DoubleRow stationary per (ktpair, head):
            # [k-part 128, pair KP, head 8, i 2, 80] (80 for 16B strides)
            vaug = big.tile([128, KP, 8, 2, 80], f8, tag="va")
            # bf16 V + ones for the first q-tile's PV (k-tiles 0..3)
            vaug_bf = big.tile([128, 4, 8, 65], bf16, tag="vab")
            ot_sb = big.tile([128, 4, t], bf16, tag="ot")      # O^T normalized

            # DMA emission order = queue order: the V-projection inputs go
            # first so the first matmul starts as early as possible.
            w_sb = {}
            for name, d_t in (("v", wvt_d), ("q", wqt_d), ("k", wkt_d)):
                w_sb[name] = wpool.tile([128, 8, G], bf16, tag=f"w{name}",
                                        name=f"w{name}")

            def load_w(name, d_t):
                # one DMA per 128-feature chunk: spreads across the 16 DMA
                # queues so the startup ramp isn't queue-limited
                wsrc = d_t[:, :].rearrange("(c p) f -> p c f", p=128)
                for cc in range(8):
                    nc.sync.dma_start(
                        out=w_sb[name][:, cc:cc + 1, :],
                        in_=wsrc[:, cc:cc + 1, :])

            load_w("v", wvt_d)
            bv_sb = wpool.tile([128, G], f32, tag="bv")
            nc.sync.dma_start(
                out=bv_sb, in_=bv_d[:].unsqueeze(0).to_broadcast((128, G))
            )
            # ones columns of V_aug
            for ii in range(2):
                nc.vector.memset(vaug[:, :, :, ii, 64], 1.0)
            nc.vector.memset(vaug_bf[:, :, :, 64], 1.0)
            maskA_sb = wpool.tile([128, 128], f32, tag="maskA")
            maskB_sb = wpool.tile([128, 256], f32, tag="maskB")
            bqk_sb = wpool.tile([128, 8], f32, tag="bqk")
            wot_sb = wpool.tile([128, 4, C], bf16, tag="wot")

            def load_late_weights():
                load_w("q", wqt_d)
                load_w("k", wkt_d)
                nc.sync.dma_start(out=bqk_sb, in_=bqk_d[:, :])
                nc.sync.dma_start(out=maskA_sb, in_=maskA_d[:, :])
                nc.sync.dma_start(out=maskB_sb, in_=maskB_d[:, :])
                nc.sync.dma_start(
                    out=wot_sb,
                    in_=wot_d[:, :].rearrange("(c p) f -> p c f", p=128),
                )

            def load_x_slice(ts):
                tsl = slice(ts * 512, (ts + 1) * 512)
                x_sl = xpool.tile([128, 8, 512], bf16, tag="xsl")
                src = xt_d[:, tsl].rearrange("(c p) t -> p c t", p=128)
                for cc in range(8):
                    nc.sync.dma_start(
                        out=x_sl[:, cc:cc + 1, :],
                        in_=src[:, cc:cc + 1, :])
                return x_sl

            def v_group(ts):
                """V projection for 4 token sub-tiles: [tok 128, feat 512]
                scattered into vaug fp8 (and vaug_bf for k-tiles 0-3)."""
                x_sl = load_x_slice(ts)
                for tsub in range(4):
                    kt_idx = ts * 4 + tsub
                    kp, ii = divmod(kt_idx, 2)
                    ps = psA.tile([128, 512], f32, tag="mm")
                    for cc in range(8):
                        nc.tensor.matmul(
                            ps,
                            lhsT=x_sl[:, cc, tsub * 128:(tsub + 1) * 128],
                            rhs=w_sb["v"][:, cc, :],
                            start=(cc == 0),
                            stop=(cc == 7),
                        )
                    ps3 = ps.rearrange("p (h d) -> p h d", d=64)
                    bv3 = bv_sb.rearrange("p (h d) -> p h d", d=64)
                    with nc.allow_low_precision(
                        reason="V stored fp8/bf16 for PV"
                    ):
                        nc.vector.tensor_add(
                            out=vaug[:, kp, :, ii, 0:64], in0=ps3, in1=bv3)
                        if ts == 0:
                            nc.vector.tensor_add(
                                out=vaug_bf[:, kt_idx, :, 0:64],
                                in0=ps3, in1=bv3)

            def qk_group(c, ts):
                """Q^T and K^T projection tiles [feat 128, tok 512] for
                head-pair chunk c, token slice ts."""
                tsl = slice(ts * 512, (ts + 1) * 512)
                x_sl = load_x_slice(ts)
                for name, dst, bcol in (("q", qt_sb, 0), ("k", kt_sb, 4)):
                    ps = psA.tile([128, 512], f32, tag="mm")
                    for cc in range(8):
                        nc.tensor.matmul(
                            ps,
                            lhsT=w_sb[name][:, cc, c * 128:(c + 1) * 128],
                            rhs=x_sl[:, cc, :],
                            start=(cc == 0),
                            stop=(cc == 7),
                        )
                    with nc.allow_low_precision(
                        reason="Q^T/K^T stored bf16 for the PE"
                    ):
                        nc.vector.tensor_scalar_add(
                            out=dst[:, c, tsl],
                            in0=ps,
                            scalar1=bqk_sb[:, bcol + c:bcol + c + 1],
                        )

            def outproj_group(tt, of):
                """Partial output projection [tok 128, outfeat 512]."""
                ps = psA.tile([128, 512], f32, tag="mm")
                for fc in range(4):
                    nc.tensor.matmul(
                        ps,
                        lhsT=ot_sb[:, fc, tt * 128:(tt + 1) * 128],
                        rhs=wot_sb[:, fc, of * 512:(of + 1) * 512],
                        start=(fc == 0),
                        stop=(fc == 3),
                    )
                o_out = small.tile([128, 512], f32, tag="oout")
                nc.vector.tensor_copy(out=o_out, in_=ps)
                nc.sync.dma_start(
                    out=out_d[tt * 128:(tt + 1) * 128,
                              of * 512:(of + 1) * 512],
                    in_=o_out,
                )

            # Softmax normalization (see module docstring).
            lnr_g = wpool.tile([33, 512], f32, tag="lnrg")
            nc.vector.memset(lnr_g, 1.0)
            norm_state = {"nf": 0, "pending": []}

            def flush_norm():
                for mul in norm_state["pending"]:
                    mul()
                norm_state["pending"] = []

            def stage_epilogue(c, qt, pv, defer=True):
                flush_norm()
                for hp in range(2):
                    nc.scalar.activation(
                        lnr_g[hp * 32:hp * 32 + 1, :], pv[hp][64:65, :],
                        func=mybir.ActivationFunctionType.Ln,
                    )
                r33 = small.tile([33, 512], bf16, tag="r33")
                nc.scalar.activation(r33, lnr_g, func=Exp, scale=-1.0)
                for hp in range(2):
                    po = hp * 64
                    osl = ot_sb[po:po + 64, c, qt * 512:(qt + 1) * 512]
                    with nc.allow_low_precision(
                        reason="O^T staged bf16; normalized in place"
                    ):
                        nc.vector.tensor_copy(out=osl, in_=pv[hp][0:64, :])
                    slot = norm_state["nf"] % 16
                    norm_state["nf"] += 1
                    nc.sync.dma_start(
                        out=rsc_d[slot:slot + 1, :],
                        in_=r33[hp * 32:hp * 32 + 1, :])
                    # both SBUF inputs of a DVE op must share the base
                    # partition: land the broadcast on the head's rows
                    bcast = small.tile([128, 512], bf16, tag="bcast")
                    nc.sync.dma_start(
                        out=bcast[po:po + 64, :],
                        in_=rsc_d[slot, :].unsqueeze(0).to_broadcast(
                            (64, 512)),
                    )

                    def mul(osl=osl, bcast=bcast, po=po):
                        nc.vector.tensor_mul(
                            out=osl, in0=osl, in1=bcast[po:po + 64, :])

                    norm_state["pending"].append(mul)
                if not defer:
                    flush_norm()

            def attention_cell(qt, c, pace):
                """Causal attention for one (q-tile, head-chunk) cell.
                Calls pace(k) after iteration k to interleave fillers."""
                pv = [
                    psPV.tile([65, 512], f32, tag="pv", name=f"pv{i}")
                    for i in range(2)
                ]

                def s_and_exp(kt, qoff, msk, mw, p_out):
                    qsl = slice(qt * 512 + qoff, (qt + 1) * 512)
                    s_ps = psS.tile([128, 1024], f32, tag="s")
                    for hp in range(2):
                        po = hp * 64
                        nc.tensor.matmul(
                            s_ps[:, hp * 512 + qoff:(hp + 1) * 512],
                            lhsT=kt_sb[po:po + 64, c,
                                       kt * 128:(kt + 1) * 128],
                            rhs=qt_sb[po:po + 64, c, qsl],
                            start=True,
                            stop=True,
                        )
                    if msk is not None:
                        for hp in range(2):
                            sl = slice(hp * 512 + qoff,
                                       hp * 512 + qoff + mw)
                            nc.vector.tensor_add(
                                out=s_ps[:, sl], in0=s_ps[:, sl], in1=msk)
                    nc.scalar.activation(
                        out=p_out.rearrange(
                            "p (h q) -> p h q", h=2)[:, :, qoff:],
                        in_=s_ps.rearrange(
                            "p (h q) -> p h q", h=2)[:, :, qoff:],
                        func=Exp,
                        scale=0.125,
                    )

                if qt == 0:
                    # 4 diagonal k-tiles, bf16 P/V, per-k-tile windows
                    for kt in range(4):
                        qoff = kt * 128
                        p_tb = ptbpool.tile([128, 1024], bf16, tag="ptb")
                        s_and_exp(kt, qoff, maskA_sb, 128, p_tb)
                        for hp in range(2):
                            h = 2 * c + hp
                            nc.tensor.matmul(
                                pv[hp][:, qoff:],
                                lhsT=vaug_bf[:, kt, h, :],
                                rhs=p_tb[:, hp * 512 + qoff:hp * 512 + 512],
                                start=(kt == 0),
                                stop=(kt == 3),
                            )
                        pace(kt, 4, 0.4)
                else:
                    npair = 2 * (qt + 1)
                    for kp in range(npair):
                        j0 = 2 * kp - 4 * qt     # diag index of even kt
                        qoff = max(j0, 0) * 128
                        p_t = ptpool.tile([128, 2, 1024], f8, tag="pt")
                        for ii in range(2):
                            if j0 >= 0:
                                msk = maskA_sb if ii == 0 else maskB_sb
                                mw = 128 if ii == 0 else 256
                            else:
                                msk, mw = None, 0
                            s_and_exp(2 * kp + ii, qoff, msk, mw,
                                      p_t[:, ii, :])
                        for hp in range(2):
                            h = 2 * c + hp
                            nc.tensor.matmul(
                                pv[hp][:, qoff:],
                                lhsT=vaug[:, kp, h, :, 0:65],
                                rhs=p_t[:, :, hp * 512 + qoff:
                                        hp * 512 + 512],
                                start=(kp == 0),
                                stop=(kp == npair - 1),
                                perf_mode=DR,
                            )
                        pace(kp, npair, 1.2)
                return pv

            # ---- main loop ----
            # Cells (q-tile, head-chunk) run q-tile-major ascending (cell
            # (qt, c) needs K^T/V for all k-tiles <= 4qt+3, so projections
            # must lead the q-tile frontier). Projection fillers carry
            # deadlines (the cell that first reads them) and are paced by
            # a PE-deficit credit so TensorE stays dense (HAM stays at
            # 2.4 GHz). Output-projection groups are PARKED until the
            # final q-tile row: that row is the most ACT-bound and has no
            # projection fillers left, so it gets the out-proj work.
            v_group(0)
            load_late_weights()
            qk_group(0, 0)

            cells = [(qt, c) for qt in range(TS) for c in range(4)]
            cell_of = {cell: i for i, cell in enumerate(cells)}

            # filler entries: (deadline_cell or None, pe_us, fn)
            fillers = []
            for ts in range(1, TS):
                fillers.append((cell_of[(ts, 0)], 3.5,
                                lambda ts=ts: v_group(ts)))
            for ts in range(TS):
                for c in range(4):
                    if (ts, c) == (0, 0):
                        continue     # prologue emits it
                    fillers.append((cell_of[(ts, c)], 3.5,
                                    lambda c=c, ts=ts: qk_group(c, ts)))
            fillers.sort(key=lambda f: f[0])
            parked = []

            state = {"credit": 0.0}

            def emit_one():
                _, cost, fn = fillers.pop(0)
                fn()
                state["credit"] -= cost

            def emit_due(idx):
                while fillers and fillers[0][0] is not None \
                        and fillers[0][0] <= idx:
                    emit_one()

            for idx, (qt, c) in enumerate(cells):
                if (qt, c) == (TS - 1, 0):
                    fillers.extend(parked)
                    parked = []
                emit_due(idx)

                def pace(k, n, deficit_us, idx=idx):
                    state["credit"] += deficit_us
                    while fillers and state["credit"] >= fillers[0][1]:
                        emit_one()

                pv = attention_cell(qt, c, pace)
                # one filler between the last PV and the epilogue hides
                # the normalization latency from the PE
                if fillers:
                    emit_one()
                last_cell = idx == len(cells) - 1
                stage_epilogue(c, qt, pv, defer=(c < 3) and not last_cell)
                if c == 3:
                    flush_norm()
                    for tt in range(qt * 4, qt * 4 + 4):
                        for of in range(2):
                            fillers.append(
                                (None, 0.95, lambda tt=tt, of=of:
                                 outproj_group(tt, of)))
            # tail: the final row's out-proj
            fillers.extend(parked)
            while fillers:
                emit_one()
            flush_norm()

    if split_waits:
        _split_excess_waits(nc)
    return nc


# ---------------------------------------------------------------------------
# Host side
# ---------------------------------------------------------------------------

_NC_CACHE = {}


def _get_nc(t=T):
    if t not in _NC_CACHE:
        _NC_CACHE[t] = build_nc(t)
    return _NC_CACHE[t]


_MASK_CACHE = None


def make_masks():
    global _MASK_CACHE
    if _MASK_CACHE is not None:
        return _MASK_CACHE
    k = np.arange(128)[:, None]
    q = np.arange(128)[None, :]
    tri = np.where(k <= q, 0.0, MASK_NEG).astype(np.float32)
    full = np.full((128, 128), MASK_NEG, np.float32)
    _MASK_CACHE = (tri, np.concatenate([full, tri], axis=1))
    return _MASK_CACHE


def core_inputs(x, Wq, bq, Wk, bk, Wv, bv, Wo, core):
    """Build the input map for one core (batch b, head group g)."""
    b, g = divmod(core, 2)
    gs = slice(g * G, (g + 1) * G)
    maskA, maskB = make_masks()
    xt = x[b].T.astype(BF16)            # [C, T]
    bqk = np.concatenate(
        [bq[gs].reshape(4, 128).T, bk[gs].reshape(4, 128).T], axis=1
    ).astype(np.float32)                                      # [128, 8]
    return {
        "xt": xt,
        "wqt": Wq[gs, :].T.astype(BF16),                      # [C, G]
        "wkt": Wk[gs, :].T.astype(BF16),
        "wvt": Wv[gs, :].T.astype(BF16),
        "wot": Wo[:, gs].T.astype(BF16),                      # [G, C]
        "bqk": bqk, "bv": bv[gs].astype(np.float32),
        "maskA": maskA, "maskB": maskB,
    }


def kernel(x, Wq, bq, Wk, bk, Wv, bv, Wo, bo, _trace=False):
    x = np.asarray(x, dtype=np.float32)
    nc = _get_nc(T)
    in_maps = [
        core_inputs(x, Wq, bq, Wk, bk, Wv, bv, Wo, c) for c in range(N_CORES)
    ]
    res = run_bass_kernel_spmd(nc, in_maps, list(range(N_CORES)), trace=_trace)
    out = np.empty((B, T, C), dtype=np.float32)
    bo = np.asarray(bo, dtype=np.float32)
    for b in range(B):
        out[b] = res.results[2 * b]["out"] + res.results[2 * b + 1]["out"]
        out[b] += bo[None, :]
    kernel.last_results = res
    return out


# revision 32
# speedup vs baseline: 1.2439x; 1.2439x over previous
"""Self-contained Trainium2 (Bass/Tile) multi-head attention kernel.

Problem: nn_MultiHeadAttention (B=4, T=2048, C=1024, H=16 heads, D=64),
fp32, causal, torch-Linear-style projections (y = x @ W.T + b).

Sharding (8 NeuronCores): data-parallel over B (4) x tensor-parallel over
head-groups (2 groups of 8 heads). Core c handles batch c//2, head group
c%2. Each core computes Q/K/V projections for its 512 features, causal
attention for its 8 heads, and a partial output projection
(O_group @ Wo[:, group].T). The host sums the two partials per batch and
adds bo.

v4 structure (measured rationale in comments):
  - Projections stay bf16: a DoubleRow fp8 matmul streams output columns
    at the same 1 col/cycle as bf16 (it doubles contraction per *pass*,
    not column rate), and the dual-fp8 residual scheme needs 12 passes
    vs bf16's 8 -- a net loss. bf16 LDWEIGHTS hide behind the previous
    matmul via the PE's background weight buffer.
  - PV runs fp8 DoubleRow over 256-token k-pairs for q-tiles >= 1: one
    240ns DR matmul replaces two 216ns bf16 ones (contraction 256 in one
    pass), with the V+ones stationary keeping softmax denominators in
    PSUM row 64. Q-tile 0 stays bf16: early queries average few keys, so
    fp8 P/V noise doesn't average out there (fp8 anywhere else that
    feeds a long random-walk sum would land ~3.6% error on the output).
  - The attention loop is q-tile-major (qt, then head-chunk): projection
    filler deadlines then spread across the whole span instead of
    piling into chunk 0, which keeps TensorE dense enough that the HAM
    clock gate stays at full rate; and each q-tile's output projection
    overlaps the next q-tile's attention.
  - Softmax normalization: r = exp(-ln(sums)) on ScalarE (one table set
    has both; the DVE reciprocal ops fail codegen on this walrus),
    partition-broadcast via a DRAM bounce, applied on DVE. The
    multiplies are deferred one attention cell so the strict-FIFO DVE
    never parks on the bounce DMA semaphore and starves the PE.
  - Causal masks are added in place on the S PSUM over exactly the
    128/256 columns that need them (maskA = tri, maskB = full|tri for
    the odd half of a diagonal k-pair).

On-device layout (per core), nothing ever needs an on-chip transpose:
  - Q^T, K^T: [feat 512, tok T] bf16 (head pair per 128-chunk); the two
    heads' S matmuls run concurrently via PE row tiling (K=64).
  - P^T = exp(0.125 * S^T + mask) (ScalarE) -> fp8 pair tile / bf16
  - PV: [65, q] += matmul(lhsT=V_aug, rhs=P^T): rows 0..63 =
    unnormalized O^T, row 64 = softmax denominators
  - out partial [tok, C] = matmul(lhsT=O^T chunks, rhs=Wo_g^T chunks)
"""

import numpy as np
import ml_dtypes

import bass_rust
import concourse.bass as bass
import concourse.mybir as mybir
import concourse.tile as tile
from concourse.bass_utils import run_bass_kernel_spmd
from concourse.vector_clock import ScopedClock

BF16 = ml_dtypes.bfloat16

B, T, C, H, D = 4, 2048, 1024, 16, 64
G = C // 2          # features per head group (8 heads x 64)
N_CORES = 8
MASK_NEG = -800.0   # pre-scale; exp(0.125 * (s - 800)) == 0 for |s| < 30

# ---------------------------------------------------------------------------
# The walrus build in this container rejects instructions carrying more than
# a couple of sync waits ("Too many sync wait commands"). Tile's kernel-tail
# drain aggregates one wait per live semaphore. Split them into individual
# SP wait instructions (program order on SP preserves the semantics).
# ---------------------------------------------------------------------------


def _patched_drain_and_barrier(self, tick_clock, wait_clock):
    nc = self.nc
    drain_inst = nc.sync.drain()
    wait_clock.add_sem_waits(
        drain_inst.ins, ScopedClock({None: tick_clock.global_clock})
    )
    si = drain_inst.ins.sync_info
    waits = list(si.on_wait) if si is not None else []
    if waits:
        drain_inst.ins.sync_info = bass_rust.SyncInfo(
            on_wait=[], on_update=list(si.on_update)
        )
        assert self.sems is not None
        by_name = {h.name: h for h in self.sems.allocated().values()}
        for w in waits:
            assert w.wait_mode == "sem-ge-imm", w
            nc.sync.wait_ge(by_name[w.ant_name], w.wait_value)

    nc.all_engine_barrier()
    assert self.sems is not None
    popped = nc._tile_sem_poison_stack.pop()
    assert popped is self._sem_poison
    nc.clear_and_free_semaphores(list(self.sems.allocated().values()))
    nc.all_engine_barrier()


tile.TileContext._drain_and_barrier = _patched_drain_and_barrier


def _split_excess_waits(nc, max_waits=1):
    """Hoist surplus sync waits into standalone same-engine EventSemaphore
    instructions placed right before the owner (this walrus encodes at most
    one wait per instruction)."""
    n = 0
    for fn in nc.m.functions:
        for blk in fn.blocks:
            new_insts = []
            for inst in blk.instructions:
                si = inst.sync_info
                waits = list(si.on_wait) if si is not None else []
                if len(waits) > max_waits:
                    for w in waits[:-max_waits]:
                        ev = mybir.InstEventSemaphore(
                            name=f"I-wsplit-{n}", ins=[], outs=[]
                        )
                        n += 1
                        ev.engine = inst.engine
                        ev.sync_info = bass_rust.SyncInfo(
                            on_wait=[w], on_update=[]
                        )
                        new_insts.append(ev)
                    inst.sync_info = bass_rust.SyncInfo(
                        on_wait=waits[-max_waits:], on_update=list(si.on_update)
                    )
                new_insts.append(inst)
            blk.instructions = new_insts


# ---------------------------------------------------------------------------
# Kernel builder (per-core program; same program on all 8 cores)
# ---------------------------------------------------------------------------

def build_nc(t=T, split_waits=True):
    f32 = mybir.dt.float32
    bf16 = mybir.dt.bfloat16
    f8 = mybir.dt.float8e4
    Exp = mybir.ActivationFunctionType.Exp
    DR = mybir.MatmulPerfMode.DoubleRow

    assert t % 512 == 0
    TS = t // 512            # 512-token slices (q-tiles)
    TK = t // 128            # 128-token k-tiles
    KP = TK // 2             # 256-token k-pairs

    nc = bass.Bass()
    xt_d = nc.dram_tensor("xt", [C, t], bf16, kind="ExternalInput")
    wqt_d = nc.dram_tensor("wqt", [C, G], bf16, kind="ExternalInput")
    wkt_d = nc.dram_tensor("wkt", [C, G], bf16, kind="ExternalInput")
    wvt_d = nc.dram_tensor("wvt", [C, G], bf16, kind="ExternalInput")
    wot_d = nc.dram_tensor("wot", [G, C], bf16, kind="ExternalInput")
    bqk_d = nc.dram_tensor("bqk", [128, 8], f32, kind="ExternalInput")
    bv_d = nc.dram_tensor("bv", [G], f32, kind="ExternalInput")
    # maskA = tri(128); maskB = [full -800 (128) | tri (128)]
    maskA_d = nc.dram_tensor("maskA", [128, 128], f32, kind="ExternalInput")
    maskB_d = nc.dram_tensor("maskB", [128, 256], f32, kind="ExternalInput")
    out_d = nc.dram_tensor("out", [t, C], f32, kind="ExternalOutput")
    rsc_d = nc.dram_tensor("rscratch", [16, 512], bf16, kind="ExternalOutput")

    with tile.TileContext(nc) as tc:
        with (
            tc.tile_pool(name="big", bufs=1) as big,
            tc.tile_pool(name="weights", bufs=1) as wpool,
            tc.tile_pool(name="xsl", bufs=3) as xpool,
            tc.tile_pool(name="pt", bufs=4) as ptpool,
            tc.tile_pool(name="ptb", bufs=3) as ptbpool,
            tc.tile_pool(name="small", bufs=4) as small,
            tc.tile_pool(name="psA", bufs=2, space="PSUM") as psA,
            tc.tile_pool(name="psS", bufs=2, space="PSUM") as psS,
            tc.tile_pool(name="psPV", bufs=2, space="PSUM") as psPV,
        ):
            # ---- persistent SBUF tensors ----
            qt_sb = big.tile([128, 4, t], bf16, tag="qt")      # Q^T
            kt_sb = big.tile([128, 4, t], bf16, tag="kt")      # K^T
            # fp8 V + ones, DoubleRow stationary per (ktpair, head):
            # [k-part 128, pair KP, head 8, i 2, 80] (80 for 16B strides)
            vaug = big.tile([128, KP, 8, 2, 80], f8, tag="va")
            # bf16 V + ones for the first q-tile's PV (k-tiles 0..3)
            vaug_bf = big.tile([128, 4, 8, 65], bf16, tag="vab")
            ot_sb = big.tile([128, 4, t], bf16, tag="ot")      # O^T normalized

            # DMA emission order = queue order: the V-projection inputs go
            # first so the first matmul starts as early as possible.
            w_sb = {}
            for name, d_t in (("v", wvt_d), ("q", wqt_d), ("k", wkt_d)):
                w_sb[name] = wpool.tile([128, 8, G], bf16, tag=f"w{name}",
                                        name=f"w{name}")

            def load_w(name, d_t):
                wsrc = d_t[:, :].rearrange("(c p) f -> p c f", p=128)
                for qtr in range(4):
                    nc.sync.dma_start(
                        out=w_sb[name][:, 2 * qtr:2 * qtr + 2, :],
                        in_=wsrc[:, 2 * qtr:2 * qtr + 2, :])

            load_w("v", wvt_d)
            bv_sb = wpool.tile([128, G], f32, tag="bv")
            nc.sync.dma_start(
                out=bv_sb, in_=bv_d[:].unsqueeze(0).to_broadcast((128, G))
            )
            # ones columns of V_aug
            for ii in range(2):
                nc.vector.memset(vaug[:, :, :, ii, 64], 1.0)
            nc.vector.memset(vaug_bf[:, :, :, 64], 1.0)
            maskA_sb = wpool.tile([128, 128], f32, tag="maskA")
            maskB_sb = wpool.tile([128, 256], f32, tag="maskB")
            bqk_sb = wpool.tile([128, 8], f32, tag="bqk")
            wot_sb = wpool.tile([128, 4, C], bf16, tag="wot")

            def load_late_weights():
                load_w("q", wqt_d)
                load_w("k", wkt_d)
                nc.sync.dma_start(out=bqk_sb, in_=bqk_d[:, :])
                nc.sync.dma_start(out=maskA_sb, in_=maskA_d[:, :])
                nc.sync.dma_start(out=maskB_sb, in_=maskB_d[:, :])
                nc.sync.dma_start(
                    out=wot_sb,
                    in_=wot_d[:, :].rearrange("(c p) f -> p c f", p=128),
                )

            def load_x_slice(ts):
                tsl = slice(ts * 512, (ts + 1) * 512)
                x_sl = xpool.tile([128, 8, 512], bf16, tag="xsl")
                src = xt_d[:, tsl].rearrange("(c p) t -> p c t", p=128)
                for qtr in range(4):
                    nc.sync.dma_start(
                        out=x_sl[:, 2 * qtr:2 * qtr + 2, :],
                        in_=src[:, 2 * qtr:2 * qtr + 2, :])
                return x_sl

            def v_part(ts, tsub, x_sl):
                """V projection of one 128-token sub-tile -> vaug."""
                kt_idx = ts * 4 + tsub
                kp, ii = divmod(kt_idx, 2)
                ps = psA.tile([128, 512], f32, tag="mm")
                for cc in range(8):
                    nc.tensor.matmul(
                        ps,
                        lhsT=x_sl[:, cc, tsub * 128:(tsub + 1) * 128],
                        rhs=w_sb["v"][:, cc, :],
                        start=(cc == 0),
                        stop=(cc == 7),
                    )
                ps3 = ps.rearrange("p (h d) -> p h d", d=64)
                bv3 = bv_sb.rearrange("p (h d) -> p h d", d=64)
                with nc.allow_low_precision(
                    reason="V stored fp8/bf16 for PV"
                ):
                    nc.vector.tensor_add(
                        out=vaug[:, kp, :, ii, 0:64], in0=ps3, in1=bv3)
                    if ts == 0:
                        nc.vector.tensor_add(
                            out=vaug_bf[:, kt_idx, :, 0:64],
                            in0=ps3, in1=bv3)

            def v_group(ts):
                """V projection filler head unit: loads x, does sub-tile 0,
                returns the other 3 sub-tiles as follow-up filler units
                (x_sl stays valid: only fillers allocate xpool tiles, and
                follow-ups go to the FRONT of the filler queue)."""
                x_sl = load_x_slice(ts)
                v_part(ts, 0, x_sl)
                return [(0.9, lambda tsub=tsub: v_part(ts, tsub, x_sl))
                        for tsub in (1, 2, 3)]

            def qk_part(c, ts, x_sl, name, dst, bcol):
                tsl = slice(ts * 512, (ts + 1) * 512)
                ps = psA.tile([128, 512], f32, tag="mm")
                for cc in range(8):
                    nc.tensor.matmul(
                        ps,
                        lhsT=w_sb[name][:, cc, c * 128:(c + 1) * 128],
                        rhs=x_sl[:, cc, :],
                        start=(cc == 0),
                        stop=(cc == 7),
                    )
                with nc.allow_low_precision(
                    reason="Q^T/K^T stored bf16 for the PE"
                ):
                    nc.vector.tensor_scalar_add(
                        out=dst[:, c, tsl],
                        in0=ps,
                        scalar1=bqk_sb[:, bcol + c:bcol + c + 1],
                    )

            def qk_group(c, ts):
                """Q/K projection filler head unit (see v_group)."""
                x_sl = load_x_slice(ts)
                qk_part(c, ts, x_sl, "q", qt_sb, 0)
                return [(1.75,
                         lambda: qk_part(c, ts, x_sl, "k", kt_sb, 4))]

            def outproj_group(tt, of):
                """Partial output projection [tok 128, outfeat 512]."""
                ps = psA.tile([128, 512], f32, tag="mm")
                for fc in range(4):
                    nc.tensor.matmul(
                        ps,
                        lhsT=ot_sb[:, fc, tt * 128:(tt + 1) * 128],
                        rhs=wot_sb[:, fc, of * 512:(of + 1) * 512],
                        start=(fc == 0),
                        stop=(fc == 3),
                    )
                o_out = small.tile([128, 512], f32, tag="oout")
                nc.vector.tensor_copy(out=o_out, in_=ps)
                nc.sync.dma_start(
                    out=out_d[tt * 128:(tt + 1) * 128,
                              of * 512:(of + 1) * 512],
                    in_=o_out,
                )

            # Softmax normalization (see module docstring).
            lnr_g = wpool.tile([33, 512], f32, tag="lnrg")
            nc.vector.memset(lnr_g, 1.0)
            norm_state = {"nf": 0, "pending": []}

            def flush_norm():
                for mul in norm_state["pending"]:
                    mul()
                norm_state["pending"] = []

            def stage_epilogue(c, qt, pv, defer=True):
                flush_norm()
                for hp in range(2):
                    nc.scalar.activation(
                        lnr_g[hp * 32:hp * 32 + 1, :], pv[hp][64:65, :],
                        func=mybir.ActivationFunctionType.Ln,
                    )
                r33 = small.tile([33, 512], bf16, tag="r33")
                nc.scalar.activation(r33, lnr_g, func=Exp, scale=-1.0)
                for hp in range(2):
                    po = hp * 64
                    osl = ot_sb[po:po + 64, c, qt * 512:(qt + 1) * 512]
                    with nc.allow_low_precision(
                        reason="O^T staged bf16; normalized in place"
                    ):
                        nc.vector.tensor_copy(out=osl, in_=pv[hp][0:64, :])
                    slot = norm_state["nf"] % 16
                    norm_state["nf"] += 1
                    nc.sync.dma_start(
                        out=rsc_d[slot:slot + 1, :],
                        in_=r33[hp * 32:hp * 32 + 1, :])
                    # both SBUF inputs of a DVE op must share the base
                    # partition: land the broadcast on the head's rows
                    bcast = small.tile([128, 512], bf16, tag="bcast")
                    nc.sync.dma_start(
                        out=bcast[po:po + 64, :],
                        in_=rsc_d[slot, :].unsqueeze(0).to_broadcast(
                            (64, 512)),
                    )

                    def mul(osl=osl, bcast=bcast, po=po):
                        nc.vector.tensor_mul(
                            out=osl, in0=osl, in1=bcast[po:po + 64, :])

                    norm_state["pending"].append(mul)
                if not defer:
                    flush_norm()

            def attention_cell(qt, c, pace):
                """Causal attention for one (q-tile, head-chunk) cell.
                Calls pace(k) after iteration k to interleave fillers."""
                pv = [
                    psPV.tile([65, 512], f32, tag="pv", name=f"pv{i}")
                    for i in range(2)
                ]

                def s_and_exp(kt, qoff, msk, mw, p_out):
                    qsl = slice(qt * 512 + qoff, (qt + 1) * 512)
                    s_ps = psS.tile([128, 1024], f32, tag="s")
                    for hp in range(2):
                        po = hp * 64
                        nc.tensor.matmul(
                            s_ps[:, hp * 512 + qoff:(hp + 1) * 512],
                            lhsT=kt_sb[po:po + 64, c,
                                       kt * 128:(kt + 1) * 128],
                            rhs=qt_sb[po:po + 64, c, qsl],
                            start=True,
                            stop=True,
                        )
                    if msk is not None:
                        for hp in range(2):
                            sl = slice(hp * 512 + qoff,
                                       hp * 512 + qoff + mw)
                            nc.vector.tensor_add(
                                out=s_ps[:, sl], in0=s_ps[:, sl], in1=msk)
                    nc.scalar.activation(
                        out=p_out.rearrange(
                            "p (h q) -> p h q", h=2)[:, :, qoff:],
                        in_=s_ps.rearrange(
                            "p (h q) -> p h q", h=2)[:, :, qoff:],
                        func=Exp,
                        scale=0.125,
                    )

                if qt == 0:
                    # 4 diagonal k-tiles, bf16 P/V, per-k-tile windows
                    for kt in range(4):
                        qoff = kt * 128
                        p_tb = ptbpool.tile([128, 1024], bf16, tag="ptb")
                        s_and_exp(kt, qoff, maskA_sb, 128, p_tb)
                        for hp in range(2):
                            h = 2 * c + hp
                            nc.tensor.matmul(
                                pv[hp][:, qoff:],
                                lhsT=vaug_bf[:, kt, h, :],
                                rhs=p_tb[:, hp * 512 + qoff:hp * 512 + 512],
                                start=(kt == 0),
                                stop=(kt == 3),
                            )
                        pace(kt, 4, 0.4)
                else:
                    npair = 2 * (qt + 1)
                    for kp in range(npair):
                        j0 = 2 * kp - 4 * qt     # diag index of even kt
                        qoff = max(j0, 0) * 128
                        p_t = ptpool.tile([128, 2, 1024], f8, tag="pt")
                        for ii in range(2):
                            if j0 >= 0:
                                msk = maskA_sb if ii == 0 else maskB_sb
                                mw = 128 if ii == 0 else 256
                            else:
                                msk, mw = None, 0
                            s_and_exp(2 * kp + ii, qoff, msk, mw,
                                      p_t[:, ii, :])
                        for hp in range(2):
                            h = 2 * c + hp
                            nc.tensor.matmul(
                                pv[hp][:, qoff:],
                                lhsT=vaug[:, kp, h, :, 0:65],
                                rhs=p_t[:, :, hp * 512 + qoff:
                                        hp * 512 + 512],
                                start=(kp == 0),
                                stop=(kp == npair - 1),
                                perf_mode=DR,
                            )
                        pace(kp, npair, 1.2)
                return pv

            # ---- main loop ----
            # Cells (q-tile, head-chunk) run q-tile-major ascending (cell
            # (qt, c) needs K^T/V for all k-tiles <= 4qt+3, so projections
            # must lead the q-tile frontier). Projection fillers carry
            # deadlines (the cell that first reads them) and are paced by
            # a PE-deficit credit so TensorE stays dense (HAM stays at
            # 2.4 GHz). Output-projection groups are PARKED until the
            # final q-tile row: that row is the most ACT-bound and has no
            # projection fillers left, so it gets the out-proj work.
            for _, f in v_group(0):
                f()
            load_late_weights()
            for _, f in qk_group(0, 0):
                f()

            cells = [(qt, c) for qt in range(TS) for c in range(4)]
            cell_of = {cell: i for i, cell in enumerate(cells)}

            # filler entries: (deadline_cell or None, pe_us, fn)
            fillers = []
            for ts in range(1, TS):
                fillers.append((cell_of[(ts, 0)], 0.9,
                                lambda ts=ts: v_group(ts)))
            for ts in range(TS):
                for c in range(4):
                    if (ts, c) == (0, 0):
                        continue     # prologue emits it
                    fillers.append((cell_of[(ts, c)], 1.75,
                                    lambda c=c, ts=ts: qk_group(c, ts)))
            fillers.sort(key=lambda f: f[0])
            parked = []

            state = {"credit": 0.0}

            def emit_one():
                dl, cost, fn = fillers.pop(0)
                follow = fn()
                state["credit"] -= cost
                if follow:
                    fillers[:0] = [(dl, fc, ff) for fc, ff in follow]

            def emit_due(idx):
                while fillers and fillers[0][0] is not None \
                        and fillers[0][0] <= idx:
                    emit_one()

            for idx, (qt, c) in enumerate(cells):
                if (qt, c) == (TS - 1, 0):
                    fillers.extend(parked)
                    parked = []
                emit_due(idx)

                def pace(k, n, deficit_us, idx=idx):
                    state["credit"] += deficit_us
                    while fillers and state["credit"] >= fillers[0][1]:
                        emit_one()

                pv = attention_cell(qt, c, pace)
                # one filler between the last PV and the epilogue hides
                # the normalization latency from the PE
                if fillers:
                    emit_one()
                last_cell = idx == len(cells) - 1
                stage_epilogue(c, qt, pv, defer=(c < 3) and not last_cell)
                if c == 3:
                    flush_norm()
                    for tt in range(qt * 4, qt * 4 + 4):
                        for of in range(2):
                            fillers.append(
                                (None, 0.95, lambda tt=tt, of=of:
                                 outproj_group(tt, of)))
            # tail: the final row's out-proj
            fillers.extend(parked)
            while fillers:
                emit_one()
            flush_norm()

    if split_waits:
        _split_excess_waits(nc)
    return nc


# ---------------------------------------------------------------------------
# Host side
# ---------------------------------------------------------------------------

_NC_CACHE = {}


def _get_nc(t=T):
    if t not in _NC_CACHE:
        _NC_CACHE[t] = build_nc(t)
    return _NC_CACHE[t]


_MASK_CACHE = None


def make_masks():
    global _MASK_CACHE
    if _MASK_CACHE is not None:
        return _MASK_CACHE
    k = np.arange(128)[:, None]
    q = np.arange(128)[None, :]
    tri = np.where(k <= q, 0.0, MASK_NEG).astype(np.float32)
    full = np.full((128, 128), MASK_NEG, np.float32)
    _MASK_CACHE = (tri, np.concatenate([full, tri], axis=1))
    return _MASK_CACHE


def core_inputs(x, Wq, bq, Wk, bk, Wv, bv, Wo, core):
    """Build the input map for one core (batch b, head group g)."""
    b, g = divmod(core, 2)
    gs = slice(g * G, (g + 1) * G)
    maskA, maskB = make_masks()
    xt = x[b].T.astype(BF16)            # [C, T]
    bqk = np.concatenate(
        [bq[gs].reshape(4, 128).T, bk[gs].reshape(4, 128).T], axis=1
    ).astype(np.float32)                                      # [128, 8]
    return {
        "xt": xt,
        "wqt": Wq[gs, :].T.astype(BF16),                      # [C, G]
        "wkt": Wk[gs, :].T.astype(BF16),
        "wvt": Wv[gs, :].T.astype(BF16),
        "wot": Wo[:, gs].T.astype(BF16),                      # [G, C]
        "bqk": bqk, "bv": bv[gs].astype(np.float32),
        "maskA": maskA, "maskB": maskB,
    }


def kernel(x, Wq, bq, Wk, bk, Wv, bv, Wo, bo, _trace=False):
    x = np.asarray(x, dtype=np.float32)
    nc = _get_nc(T)
    in_maps = [
        core_inputs(x, Wq, bq, Wk, bk, Wv, bv, Wo, c) for c in range(N_CORES)
    ]
    res = run_bass_kernel_spmd(nc, in_maps, list(range(N_CORES)), trace=_trace)
    out = np.empty((B, T, C), dtype=np.float32)
    bo = np.asarray(bo, dtype=np.float32)
    for b in range(B):
        out[b] = res.results[2 * b]["out"] + res.results[2 * b + 1]["out"]
        out[b] += bo[None, :]
    kernel.last_results = res
    return out
